# revision 23
# baseline (speedup 1.0000x reference)
"""Trainium2 Bass kernel for nn_ASAMLayer (local-window sparse attention layer).

Sharding: token-parallel across 8 cores. 4096 tokens total -> 512 own tokens
per core, plus a 128-token halo on each side (within-batch, zero-padded at
batch edges) so the WINDOW=128 local attention needs no collectives.

On-chip layout: feature-major ("transposed") activations for all GEMMs.
LayerNorm gains/biases are folded into the following weight matrices on the
host (exact), softmax runs without max-subtraction (scores are bounded),
row-sums come free from the activation engine's accum_out, and the band mask
is a precomputed additive -1e6 tensor applied to scores in PSUM.
"""

import sys

import numpy as np

sys.path.insert(0, "/opt/trn_rl_repo")

import ml_dtypes  # noqa: E402

import concourse.bass as bass  # noqa: E402
from concourse import bacc  # noqa: E402
import concourse.mybir as mybir  # noqa: E402
import concourse.tile as tile  # noqa: E402
from concourse.bass_utils import run_bass_kernel_spmd  # noqa: E402


B, S, D = 2, 2048, 1024
H, DH = 16, 64
INNER = H * DH          # 1024
FF = 4 * D              # 4096
WINDOW = 128
EPS = 1e-5
SCALE = DH ** -0.5

NCORES = 8
OWN = (B * S) // NCORES          # 512 own tokens per core
HALO = WINDOW                    # 128
TLOC = OWN + 2 * HALO            # 768 local rows (halo'd)
P = 128
NQT = OWN // P                   # 4 query tiles
NKT = TLOC // P                  # 6 key tiles
KW = 3 * P                       # 384-wide key window per query tile

F32 = mybir.dt.float32
BF16 = mybir.dt.bfloat16
BF = ml_dtypes.bfloat16

MASK_NEG = -1.0e6


def _build_nc():
    nc = bacc.Bacc()

    x_s = nc.declare_dram_parameter("x_s", [TLOC, D], F32, isOutput=False)
    maskin = nc.declare_dram_parameter("maskin", [NQT, P, KW], F32, isOutput=False)
    wq_t = nc.declare_dram_parameter("wq_t", [8, P, 8, P], BF16, isOutput=False)
    wk_t = nc.declare_dram_parameter("wk_t", [8, P, 8, P], BF16, isOutput=False)
    wv_t = nc.declare_dram_parameter("wv_t", [P, 8, INNER], BF16, isOutput=False)
    wout_t = nc.declare_dram_parameter("wout_t", [8, P, 8, P], BF16, isOutput=False)
    wff1_t = nc.declare_dram_parameter("wff1_t", [32, P, 8, P], BF16, isOutput=False)
    wff2_t = nc.declare_dram_parameter("wff2_t", [8, P, 32, P], BF16, isOutput=False)
    bq_t = nc.declare_dram_parameter("bq_t", [P, 8], F32, isOutput=False)
    bk_t = nc.declare_dram_parameter("bk_t", [P, 8], F32, isOutput=False)
    bout_t = nc.declare_dram_parameter("bout_t", [P, 8], F32, isOutput=False)
    bff1_t = nc.declare_dram_parameter("bff1_t", [P, 32], F32, isOutput=False)
    bff2_t = nc.declare_dram_parameter("bff2_t", [P, 8], F32, isOutput=False)
    y = nc.declare_dram_parameter("y", [OWN, D], F32, isOutput=True)

    with tile.TileContext(nc) as tc:
        _emit(tc, nc, x_s, maskin, wq_t, wk_t, wv_t, wout_t, wff1_t, wff2_t,
              bq_t, bk_t, bout_t, bff1_t, bff2_t, y)
    nc.finalize()
    return nc


def _layernorm_tile(nc, pool, x_ap, out_bf16, eps_ap):
    """out_bf16 = (x - mean(x)) / sqrt(var(x) + EPS), bf16. x_ap [P, D] fp32."""
    xg = x_ap.rearrange("p (s f) -> p s f", f=512)
    stats = pool.tile([P, 2, 6], F32, tag="ln_stats")
    for s in range(2):
        nc.vector.bn_stats(out=stats[:, s, :], in_=xg[:, s, :])
    mv = pool.tile([P, 2], F32, tag="ln_mv")
    nc.vector.bn_aggr(out=mv[:], in_=stats[:])
    rstd = pool.tile([P, 1], F32, tag="ln_rstd")
    nc.scalar.activation(out=rstd[:], in_=mv[:, 1:2],
                         func=mybir.ActivationFunctionType.Sqrt,
                         bias=eps_ap, scale=1.0)
    nc.vector.reciprocal(out=rstd[:], in_=rstd[:])
    nc.vector.tensor_scalar(
        out=out_bf16[:], in0=x_ap,
        scalar1=mv[:, 0:1], scalar2=rstd[:],
        op0=mybir.AluOpType.subtract, op1=mybir.AluOpType.mult)


def _emit(tc, nc, x_s, maskin, wq_t, wk_t, wv_t, wout_t, wff1_t, wff2_t,
          bq_t, bk_t, bout_t, bff1_t, bff2_t, y):
    from contextlib import ExitStack
    ctx = ExitStack()
    Gelu = mybir.ActivationFunctionType.Gelu
    Exp = mybir.ActivationFunctionType.Exp
    Ident = mybir.ActivationFunctionType.Identity

    const = ctx.enter_context(tc.tile_pool(name="const", bufs=1))
    small = ctx.enter_context(tc.tile_pool(name="small", bufs=4))
    wst = ctx.enter_context(tc.tile_pool(name="wst", bufs=3))
    act = ctx.enter_context(tc.tile_pool(name="act", bufs=1))
    trans = ctx.enter_context(tc.tile_pool(name="trans", bufs=3))
    psum = ctx.enter_context(tc.tile_pool(name="psum", bufs=3, space="PSUM"))
    psum_kw = ctx.enter_context(tc.tile_pool(name="pskw", bufs=3, space="PSUM"))
    psum_small = ctx.enter_context(tc.tile_pool(name="psA", bufs=2, space="PSUM"))

    # ---- load wv (needed first), x, biases, masks ----
    wv = act.tile([P, 8, INNER], BF16, tag="big_wx2")   # 16KB/p (dies after V)
    nc.gpsimd.dma_start(out=wv[:], in_=wv_t[:])
    xt = act.tile([P, NKT, D], F32, tag="big_xh")       # 24KB/p
    for t in range(NKT):
        nc.sync.dma_start(out=xt[:, t, :], in_=x_s[t * P:(t + 1) * P, :])
    bq = const.tile([P, 8], F32, tag="bq")
    nc.gpsimd.dma_start(out=bq[:], in_=bq_t[:])
    bk = const.tile([P, 8], F32, tag="bk")
    nc.gpsimd.dma_start(out=bk[:], in_=bk_t[:])
    bout = const.tile([P, 8], F32, tag="bout")
    nc.gpsimd.dma_start(out=bout[:], in_=bout_t[:])
    bff1 = const.tile([P, 32], F32, tag="bff1")
    nc.gpsimd.dma_start(out=bff1[:], in_=bff1_t[:])
    bff2 = const.tile([P, 8], F32, tag="bff2")
    nc.gpsimd.dma_start(out=bff2[:], in_=bff2_t[:])
    eps_t = const.tile([P, 1], F32, tag="eps")
    nc.vector.memset(eps_t[:], EPS)
    maskt = act.tile([P, NQT, KW], F32, tag="mask")     # 6KB/p
    for q in range(NQT):
        nc.gpsimd.dma_start(out=maskt[:, q, :], in_=maskin[q])

    # ---- per token-tile: LN1 -> transpose -> V matmuls (PE starts early) ----
    yt = act.tile([P, 8, TLOC], BF16, tag="big_ya")     # 12KB/p
    vtok = act.tile([P, NKT, INNER], BF16, tag="vtok")  # 12KB/p
    for t in range(NKT):
        y16 = trans.tile([P, D], BF16, tag="y16t")
        _layernorm_tile(nc, small, xt[:, t, :], y16[:], eps_t[:])
        nc.sync.dma_start_transpose(yt[:, :, t * P:(t + 1) * P], y16[:])
        for half in range(2):
            ps = psum.tile([P, OWN], F32, tag="ps_big")
            sl = slice(half * 512, (half + 1) * 512)
            for k in range(8):
                nc.tensor.matmul(ps[:], yt[:, k, t * P:(t + 1) * P], wv[:, k, sl],
                                 start=(k == 0), stop=(k == 7))
            nc.vector.tensor_copy(out=vtok[:, t, sl], in_=ps[:])

    # ---- Q (own queries) and K (all local keys), feature-major ----
    qt_sb = act.tile([P, 8, OWN], BF16, tag="big_qoy")  # 8KB/p
    kt_sb = act.tile([P, 8, TLOC], BF16, tag="kt")      # 12KB/p
    for o in range(8):
        w = wst.tile([P, 8, P], BF16, tag="w_small")
        nc.sync.dma_start(out=w[:], in_=wq_t[o])
        ps = psum.tile([P, OWN], F32, tag="ps_big")
        for k in range(8):
            nc.tensor.matmul(ps[:], w[:, k, :], yt[:, k, HALO:HALO + OWN],
                             start=(k == 0), stop=(k == 7))
        nc.scalar.activation(out=qt_sb[:, o, :], in_=ps[:], func=Ident,
                             bias=bq[:, o:o + 1], scale=1.0)
    for o in range(8):
        w = wst.tile([P, 8, P], BF16, tag="w_small")
        nc.sync.dma_start(out=w[:], in_=wk_t[o])
        for half in range(2):
            ps = psum_kw.tile([P, KW], F32, tag="ps_kw")
            sl = slice(half * KW, (half + 1) * KW)
            for k in range(8):
                nc.tensor.matmul(ps[:], w[:, k, :], yt[:, k, sl],
                                 start=(k == 0), stop=(k == 7))
            nc.scalar.activation(out=kt_sb[:, o, sl], in_=ps[:], func=Ident,
                                 bias=bk[:, o:o + 1], scale=1.0)

    wout_sb = act.tile([P, 8, 8, P], BF16, tag="woutr")   # 16KB/p resident
    for o in range(8):
        nc.sync.dma_start(out=wout_sb[:, o], in_=wout_t[o])

    # ---- attention + per-qtile epilogue (out-proj, residual, LN2, Zt) ----
    avt = act.tile([P, 8, OWN], BF16, tag="big_ya")     # reuses yt slot
    x2 = act.tile([P, NQT, D], F32, tag="big_wx2")      # reuses wv slot
    z16 = act.tile([P, NQT, D], BF16, tag="big_zf")     # 8KB/p
    zt = act.tile([P, 8, OWN], BF16, tag="zt")          # 8KB/p
    for qtl in range(NQT):
        qsl = slice(qtl * P, (qtl + 1) * P)
        for i in range(8):
            for hh in range(2):
                h = 2 * i + hh
                hs = slice(hh * DH, (hh + 1) * DH)
                ksl = slice(qtl * P, qtl * P + KW)
                sc = psum_kw.tile([P, KW], F32, tag="ps_kw")
                nc.tensor.matmul(sc[:], qt_sb[hs, i, qsl], kt_sb[hs, i, ksl],
                                 start=True, stop=True)
                scm = trans.tile([P, KW], F32, tag="scm")
                nc.vector.tensor_tensor(out=scm[:], in0=sc[:],
                                        in1=maskt[:, qtl, :],
                                        op=mybir.AluOpType.add)
                ptm = trans.tile([P, KW], BF16, tag="ptm")
                ssum = small.tile([P, 1], F32, tag="ssum")
                nc.scalar.activation(out=ptm[:], in_=scm[:], func=Exp,
                                     scale=SCALE, accum_out=ssum[:])
                nc.vector.reciprocal(out=ssum[:], in_=ssum[:])
                nc.vector.tensor_scalar_mul(ptm[:], ptm[:], ssum[:])
                ptt = trans.tile([P, 3, P], BF16, tag="ptt")
                for e in range(3):
                    nc.sync.dma_start_transpose(ptt[:, e, :],
                                                ptm[:, e * P:(e + 1) * P])
                av = psum_small.tile([DH, P], F32, tag="ps_av")
                for e in range(3):
                    nc.tensor.matmul(av[:], vtok[:, qtl + e, h * DH:(h + 1) * DH],
                                     ptt[:, e, :], start=(e == 0), stop=(e == 2))
                nc.any.tensor_copy(out=avt[hs, i, qsl], in_=av[:])
        # out-projection for this qtile, then residual + LN2 + Zt
        osb_q = trans.tile([P, 8, P], BF16, tag="osbq")
        otok_q = trans.tile([P, D], BF16, tag="otokq")
        for o in range(8):
            ps = psum_kw.tile([P, KW], F32, tag="ps_kw")
            for k in range(8):
                nc.tensor.matmul(ps[:, :P], wout_sb[:, o, k, :], avt[:, k, qsl],
                                 start=(k == 0), stop=(k == 7))
            nc.scalar.activation(out=osb_q[:, o, :], in_=ps[:, :P], func=Ident,
                                 bias=bout[:, o:o + 1], scale=1.0)
            nc.sync.dma_start_transpose(otok_q[:, o * P:(o + 1) * P],
                                        osb_q[:, o, :])
            osl = slice(o * P, (o + 1) * P)
            nc.vector.tensor_tensor(out=x2[:, qtl, osl],
                                    in0=xt[:, qtl + 1, osl],
                                    in1=otok_q[:, osl],
                                    op=mybir.AluOpType.add)
        _layernorm_tile(nc, small, x2[:, qtl, :], z16[:, qtl, :], eps_t[:])
        nc.sync.dma_start_transpose(zt[:, :, qtl * P:(qtl + 1) * P],
                                    z16[:, qtl, :])

    # ---- FFN ----
    ht = act.tile([P, 32, OWN], BF16, tag="big_xh")     # reuses x slot
    for o in range(32):
        w = wst.tile([P, 8, P], BF16, tag="w_small")
        nc.scalar.dma_start(out=w[:], in_=wff1_t[o])
        ps = psum.tile([P, OWN], F32, tag="ps_big")
        for q in range(NQT):
            for k in range(8):
                nc.tensor.matmul(ps[:, q * P:(q + 1) * P], w[:, k, :],
                                 zt[:, k, q * P:(q + 1) * P],
                                 start=(k == 0), stop=(k == 7))
        nc.scalar.activation(out=ht[:, o, :], in_=ps[:], func=Gelu,
                             bias=bff1[:, o:o + 1], scale=1.0)
    fsb = act.tile([P, 8, OWN], BF16, tag="fsb")        # 8KB/p
    ftok = act.tile([P, NQT, D], BF16, tag="big_zf")    # reuses z16 slot
    yout = act.tile([P, NQT, D], F32, tag="big_qoy")    # reuses qt slot
    for o in range(8):
        w = wst.tile([P, 32, P], BF16, tag="w_ff2")
        nc.sync.dma_start(out=w[:], in_=wff2_t[o])
        ps = psum.tile([P, OWN], F32, tag="ps_big")
        for k in range(32):
            nc.tensor.matmul(ps[:], w[:, k, :], ht[:, k, :],
                             start=(k == 0), stop=(k == 31))
        nc.scalar.activation(out=fsb[:, o, :], in_=ps[:], func=Ident,
                             bias=bff2[:, o:o + 1], scale=1.0)
        nc.sync.dma_start_transpose(ftok[:, :, o * P:(o + 1) * P], fsb[:, o, :])
        osl = slice(o * P, (o + 1) * P)
        for q in range(NQT):
            nc.vector.tensor_tensor(out=yout[:, q, osl], in0=x2[:, q, osl],
                                    in1=ftok[:, q, osl],
                                    op=mybir.AluOpType.add)
    for q in range(NQT):
        nc.sync.dma_start(out=y[q * P:(q + 1) * P, :], in_=yout[:, q, :])
    ctx.close()


def _host_prep(x, ln1_g, ln1_b, w_qkv, w_out, b_out, ln2_g, ln2_b,
               w_ff1, b_ff1, w_ff2, b_ff2):
    """Fold LN affine params into weights, pre-transpose/tile, build per-core
    input maps."""
    f8 = np.float64
    wqkv_eff = (w_qkv.astype(f8) * ln1_g.astype(f8)[None, :])
    bqkv_eff = w_qkv.astype(f8) @ ln1_b.astype(f8)
    wq, wk, wv = wqkv_eff[:INNER], wqkv_eff[INNER:2 * INNER], wqkv_eff[2 * INNER:]
    bq_v, bk_v, bv_v = bqkv_eff[:INNER], bqkv_eff[INNER:2 * INNER], bqkv_eff[2 * INNER:]
    bout_eff = b_out.astype(f8) + w_out.astype(f8) @ bv_v
    wff1_eff = w_ff1.astype(f8) * ln2_g.astype(f8)[None, :]
    bff1_eff = b_ff1.astype(f8) + w_ff1.astype(f8) @ ln2_b.astype(f8)

    def lhst(w, ko, no):  # w [K, N] -> [no, 128, ko, 128] bf16 (p = K within tile)
        a = np.ascontiguousarray(
            w.reshape(ko, P, no, P).transpose(2, 1, 0, 3)).astype(BF)
        return a

    wq_t = lhst(wq.T, 8, 8)
    wk_t = lhst(wk.T, 8, 8)
    wv_t = np.ascontiguousarray(wv.T.reshape(8, P, INNER).transpose(1, 0, 2)).astype(BF)
    wout_t = lhst(w_out.astype(f8).T, 8, 8)
    wff1_t = lhst(wff1_eff.T, 8, 32)
    wff2_t = lhst(w_ff2.astype(f8).T, 32, 8)

    def colmaj(b, n):  # [n*128] -> [128, n] fp32
        return np.ascontiguousarray(b.reshape(n, P).T).astype(np.float32)

    bq_t = colmaj(bq_v, 8)
    bk_t = colmaj(bk_v, 8)
    bout_t = colmaj(bout_eff, 8)
    bff1_t = colmaj(bff1_eff, 32)
    bff2_t = colmaj(b_ff2.astype(f8), 8)

    xf = x.reshape(B * S, D).astype(np.float32)
    in_maps = []
    for c in range(NCORES):
        b = c // (NCORES // B)
        s0 = (c % (NCORES // B)) * OWN          # within-batch start of own rows
        lo, hi = s0 - HALO, s0 + OWN + HALO
        xs = np.zeros((TLOC, D), np.float32)
        clo, chi = max(lo, 0), min(hi, S)
        xs[clo - lo:chi - lo] = xf[b * S + clo:b * S + chi]
        # additive mask [NQT, 128, 384]: query r in tile qtl, key col ccol
        q_idx = s0 + np.arange(OWN)             # within-batch query positions
        mask = np.zeros((NQT, P, KW), np.float32)
        for qtl in range(NQT):
            qq = q_idx[qtl * P:(qtl + 1) * P][:, None]      # [128,1]
            kk = (s0 + qtl * P - HALO) + np.arange(KW)[None, :]
            bad = (np.abs(kk - qq) > WINDOW) | (kk < 0) | (kk >= S)
            mask[qtl][bad] = MASK_NEG
        in_maps.append(dict(
            x_s=xs, maskin=mask, wq_t=wq_t, wk_t=wk_t, wv_t=wv_t,
            wout_t=wout_t, wff1_t=wff1_t, wff2_t=wff2_t,
            bq_t=bq_t, bk_t=bk_t, bout_t=bout_t, bff1_t=bff1_t, bff2_t=bff2_t))
    return in_maps


_NC_CACHE = {}


def kernel(x, ln1_g, ln1_b, w_qkv, w_out, b_out, ln2_g, ln2_b,
           w_ff1, b_ff1, w_ff2, b_ff2, _trace=False):
    in_maps = _host_prep(x, ln1_g, ln1_b, w_qkv, w_out, b_out,
                         ln2_g, ln2_b, w_ff1, b_ff1, w_ff2, b_ff2)
    if "nc" not in _NC_CACHE:
        _NC_CACHE["nc"] = _build_nc()
    nc = _NC_CACHE["nc"]
    res = run_bass_kernel_spmd(nc, in_maps, core_ids=list(range(NCORES)),
                               trace=_trace)
    if _trace:
        _NC_CACHE["last"] = res
    out = np.concatenate([res.results[c]["y"] for c in range(NCORES)], axis=0)
    return out.reshape(B, S, D).astype(np.float32)


# revision 25
# speedup vs baseline: 1.0233x; 1.0233x over previous
"""Trainium2 Bass kernel for nn_ASAMLayer (local-window sparse attention layer).

Sharding: token-parallel across 8 cores. 4096 tokens total -> 512 own tokens
per core, plus a 128-token halo on each side (within-batch, zero-padded at
batch edges) so the WINDOW=128 local attention needs no collectives.

On-chip layout: feature-major ("transposed") activations for all GEMMs.
LayerNorm gains/biases are folded into the following weight matrices on the
host (exact), softmax runs without max-subtraction (scores are bounded),
row-sums come free from the activation engine's accum_out, and the band mask
is a precomputed additive -1e6 tensor applied to scores in PSUM.
"""

import sys

import numpy as np

sys.path.insert(0, "/opt/trn_rl_repo")

import ml_dtypes  # noqa: E402

import concourse.bass as bass  # noqa: E402
from concourse import bacc  # noqa: E402
import concourse.mybir as mybir  # noqa: E402
import concourse.tile as tile  # noqa: E402
from concourse.bass_utils import run_bass_kernel_spmd  # noqa: E402

# Pin Exp and Ln to the joint set and Gelu to its anchor set so the act-table
# load pass emits few loads instead of thrashing per qtile. Set indices are
# preserved (unwanted sets are emptied, not removed).
import concourse.bacc as _bacc_mod  # noqa: E402
import concourse.hw_specs as _hw_specs  # noqa: E402

_orig_get_tables = _hw_specs.get_activation_tables


def _pinned_tables(module_arch):
    t = dict(_orig_get_tables(module_arch))
    keep = {"natural_log_exp_and_others", "gelu_and_others"}
    drop = {mybir.ActivationFunctionType.Exp,
            mybir.ActivationFunctionType.Ln,
            mybir.ActivationFunctionType.Gelu}
    return {name: (fns if name in keep else {f for f in fns if f not in drop})
            for name, fns in t.items()}


_bacc_mod.get_activation_tables = _pinned_tables


B, S, D = 2, 2048, 1024
H, DH = 16, 64
INNER = H * DH          # 1024
FF = 4 * D              # 4096
WINDOW = 128
EPS = 1e-5
SCALE = DH ** -0.5

NCORES = 8
OWN = (B * S) // NCORES          # 512 own tokens per core
HALO = WINDOW                    # 128
TLOC = OWN + 2 * HALO            # 768 local rows (halo'd)
P = 128
NQT = OWN // P                   # 4 query tiles
NKT = TLOC // P                  # 6 key tiles
KW = 3 * P                       # 384-wide key window per query tile

F32 = mybir.dt.float32
BF16 = mybir.dt.bfloat16
BF = ml_dtypes.bfloat16

MASK_NEG = -1.0e6


def _build_nc():
    nc = bacc.Bacc()

    x_s = nc.declare_dram_parameter("x_s", [TLOC, D], F32, isOutput=False)
    maskin = nc.declare_dram_parameter("maskin", [NQT, P, KW], F32, isOutput=False)
    wq_t = nc.declare_dram_parameter("wq_t", [8, P, 8, P], BF16, isOutput=False)
    wk_t = nc.declare_dram_parameter("wk_t", [8, P, 8, P], BF16, isOutput=False)
    wv_t = nc.declare_dram_parameter("wv_t", [P, 8, INNER], BF16, isOutput=False)
    wout_t = nc.declare_dram_parameter("wout_t", [8, P, 8, P], BF16, isOutput=False)
    wff1_t = nc.declare_dram_parameter("wff1_t", [32, P, 8, P], BF16, isOutput=False)
    wff2_t = nc.declare_dram_parameter("wff2_t", [8, P, 32, P], BF16, isOutput=False)
    bq_t = nc.declare_dram_parameter("bq_t", [P, 8], F32, isOutput=False)
    bk_t = nc.declare_dram_parameter("bk_t", [P, 8], F32, isOutput=False)
    bout_t = nc.declare_dram_parameter("bout_t", [P, 8], F32, isOutput=False)
    bff1_t = nc.declare_dram_parameter("bff1_t", [P, 32], F32, isOutput=False)
    bff2_t = nc.declare_dram_parameter("bff2_t", [P, 8], F32, isOutput=False)
    y = nc.declare_dram_parameter("y", [OWN, D], F32, isOutput=True)

    with tile.TileContext(nc) as tc:
        _emit(tc, nc, x_s, maskin, wq_t, wk_t, wv_t, wout_t, wff1_t, wff2_t,
              bq_t, bk_t, bout_t, bff1_t, bff2_t, y)
    nc.finalize()
    return nc


def _layernorm_tile(nc, pool, x_ap, out_bf16, eps_ap):
    """out_bf16 = (x - mean(x)) / sqrt(var(x) + EPS), bf16. x_ap [P, D] fp32."""
    xg = x_ap.rearrange("p (s f) -> p s f", f=512)
    stats = pool.tile([P, 2, 6], F32, tag="ln_stats")
    for s in range(2):
        nc.vector.bn_stats(out=stats[:, s, :], in_=xg[:, s, :])
    mv = pool.tile([P, 2], F32, tag="ln_mv")
    nc.vector.bn_aggr(out=mv[:], in_=stats[:])
    rstd = pool.tile([P, 1], F32, tag="ln_rstd")
    nc.scalar.activation(out=rstd[:], in_=mv[:, 1:2],
                         func=mybir.ActivationFunctionType.Ln,
                         bias=eps_ap, scale=1.0)
    nc.scalar.activation(out=rstd[:], in_=rstd[:],
                         func=mybir.ActivationFunctionType.Exp,
                         scale=-0.5)
    nc.vector.tensor_scalar(
        out=out_bf16[:], in0=x_ap,
        scalar1=mv[:, 0:1], scalar2=rstd[:],
        op0=mybir.AluOpType.subtract, op1=mybir.AluOpType.mult)


def _emit(tc, nc, x_s, maskin, wq_t, wk_t, wv_t, wout_t, wff1_t, wff2_t,
          bq_t, bk_t, bout_t, bff1_t, bff2_t, y):
    from contextlib import ExitStack
    ctx = ExitStack()
    Gelu = mybir.ActivationFunctionType.Gelu
    Exp = mybir.ActivationFunctionType.Exp
    Ident = mybir.ActivationFunctionType.Identity

    const = ctx.enter_context(tc.tile_pool(name="const", bufs=1))
    small = ctx.enter_context(tc.tile_pool(name="small", bufs=4))
    wst = ctx.enter_context(tc.tile_pool(name="wst", bufs=3))
    act = ctx.enter_context(tc.tile_pool(name="act", bufs=1))
    trans = ctx.enter_context(tc.tile_pool(name="trans", bufs=3))
    psum = ctx.enter_context(tc.tile_pool(name="psum", bufs=3, space="PSUM"))
    psum_kw = ctx.enter_context(tc.tile_pool(name="pskw", bufs=3, space="PSUM"))
    psum_small = ctx.enter_context(tc.tile_pool(name="psA", bufs=2, space="PSUM"))

    # ---- load wv (needed first), x, biases, masks ----
    wv = act.tile([P, 8, INNER], BF16, tag="big_wx2")   # 16KB/p (dies after V)
    nc.gpsimd.dma_start(out=wv[:], in_=wv_t[:])
    xt = act.tile([P, NKT, D], F32, tag="big_xh")       # 24KB/p
    for t in range(NKT):
        nc.sync.dma_start(out=xt[:, t, :], in_=x_s[t * P:(t + 1) * P, :])
    bq = const.tile([P, 8], F32, tag="bq")
    nc.gpsimd.dma_start(out=bq[:], in_=bq_t[:])
    bk = const.tile([P, 8], F32, tag="bk")
    nc.gpsimd.dma_start(out=bk[:], in_=bk_t[:])
    bout = const.tile([P, 8], F32, tag="bout")
    nc.gpsimd.dma_start(out=bout[:], in_=bout_t[:])
    bff1 = const.tile([P, 32], F32, tag="bff1")
    nc.gpsimd.dma_start(out=bff1[:], in_=bff1_t[:])
    bff2 = const.tile([P, 8], F32, tag="bff2")
    nc.gpsimd.dma_start(out=bff2[:], in_=bff2_t[:])
    eps_t = const.tile([P, 1], F32, tag="eps")
    nc.vector.memset(eps_t[:], EPS)
    maskt = act.tile([P, NQT, KW], F32, tag="mask")     # 6KB/p
    for q in range(NQT):
        nc.gpsimd.dma_start(out=maskt[:, q, :], in_=maskin[q])

    # ---- per token-tile: LN1 -> transpose -> V matmuls (PE starts early) ----
    yt = act.tile([P, 8, TLOC], BF16, tag="big_ya")     # 12KB/p
    vtok = act.tile([P, NKT, INNER], BF16, tag="vtok")  # 12KB/p
    for t in range(NKT):
        y16 = trans.tile([P, D], BF16, tag="y16t")
        _layernorm_tile(nc, small, xt[:, t, :], y16[:], eps_t[:])
        nc.sync.dma_start_transpose(yt[:, :, t * P:(t + 1) * P], y16[:])
        for half in range(2):
            ps = psum.tile([P, OWN], F32, tag="ps_big")
            sl = slice(half * 512, (half + 1) * 512)
            for k in range(8):
                nc.tensor.matmul(ps[:], yt[:, k, t * P:(t + 1) * P], wv[:, k, sl],
                                 start=(k == 0), stop=(k == 7))
            nc.vector.tensor_copy(out=vtok[:, t, sl], in_=ps[:])

    # ---- Q (own queries) and K (all local keys), feature-major ----
    qt_sb = act.tile([P, 8, OWN], BF16, tag="big_qoy")  # 8KB/p
    kt_sb = act.tile([P, 8, TLOC], BF16, tag="kt")      # 12KB/p
    for o in range(8):
        w = wst.tile([P, 8, P], BF16, tag="w_small")
        nc.sync.dma_start(out=w[:], in_=wq_t[o])
        ps = psum.tile([P, OWN], F32, tag="ps_big")
        for k in range(8):
            nc.tensor.matmul(ps[:], w[:, k, :], yt[:, k, HALO:HALO + OWN],
                             start=(k == 0), stop=(k == 7))
        nc.scalar.activation(out=qt_sb[:, o, :], in_=ps[:], func=Ident,
                             bias=bq[:, o:o + 1], scale=1.0)
    for o in range(8):
        w = wst.tile([P, 8, P], BF16, tag="w_small")
        nc.sync.dma_start(out=w[:], in_=wk_t[o])
        for half in range(2):
            ps = psum_kw.tile([P, KW], F32, tag="ps_kw")
            sl = slice(half * KW, (half + 1) * KW)
            for k in range(8):
                nc.tensor.matmul(ps[:], w[:, k, :], yt[:, k, sl],
                                 start=(k == 0), stop=(k == 7))
            nc.scalar.activation(out=kt_sb[:, o, sl], in_=ps[:], func=Ident,
                                 bias=bk[:, o:o + 1], scale=1.0)

    wout_sb = act.tile([P, 8, 8, P], BF16, tag="woutr")   # 16KB/p resident
    for o in range(8):
        nc.sync.dma_start(out=wout_sb[:, o], in_=wout_t[o])

    # ---- attention + per-qtile epilogue (out-proj, residual, LN2, Zt) ----
    avt = act.tile([P, 8, OWN], BF16, tag="big_ya")     # reuses yt slot
    x2 = act.tile([P, NQT, D], F32, tag="big_wx2")      # reuses wv slot
    z16 = act.tile([P, NQT, D], BF16, tag="big_zf")     # 8KB/p
    zt = act.tile([P, 8, OWN], BF16, tag="zt")          # 8KB/p
    for qtl in range(NQT):
        qsl = slice(qtl * P, (qtl + 1) * P)
        for i in range(8):
            for hh in range(2):
                h = 2 * i + hh
                hs = slice(hh * DH, (hh + 1) * DH)
                ksl = slice(qtl * P, qtl * P + KW)
                sc = psum_kw.tile([P, KW], F32, tag="ps_kw")
                nc.tensor.matmul(sc[:], qt_sb[hs, i, qsl], kt_sb[hs, i, ksl],
                                 start=True, stop=True)
                scm = trans.tile([P, KW], F32, tag="scm")
                nc.vector.tensor_tensor(out=scm[:], in0=sc[:],
                                        in1=maskt[:, qtl, :],
                                        op=mybir.AluOpType.add)
                ptm = trans.tile([P, KW], BF16, tag="ptm")
                ssum = small.tile([P, 1], F32, tag="ssum")
                nc.scalar.activation(out=ptm[:], in_=scm[:], func=Exp,
                                     scale=SCALE, accum_out=ssum[:])
                nc.vector.reciprocal(out=ssum[:], in_=ssum[:])
                nc.vector.tensor_scalar_mul(ptm[:], ptm[:], ssum[:])
                ptt = trans.tile([P, 3, P], BF16, tag="ptt")
                for e in range(3):
                    nc.sync.dma_start_transpose(ptt[:, e, :],
                                                ptm[:, e * P:(e + 1) * P])
                av = psum_small.tile([DH, P], F32, tag="ps_av")
                for e in range(3):
                    nc.tensor.matmul(av[:], vtok[:, qtl + e, h * DH:(h + 1) * DH],
                                     ptt[:, e, :], start=(e == 0), stop=(e == 2))
                nc.any.tensor_copy(out=avt[hs, i, qsl], in_=av[:])
        # out-projection for this qtile, then residual + LN2 + Zt
        osb_q = trans.tile([P, 8, P], BF16, tag="osbq")
        otok_q = trans.tile([P, D], BF16, tag="otokq")
        for o in range(8):
            ps = psum_kw.tile([P, KW], F32, tag="ps_kw")
            for k in range(8):
                nc.tensor.matmul(ps[:, :P], wout_sb[:, o, k, :], avt[:, k, qsl],
                                 start=(k == 0), stop=(k == 7))
            nc.scalar.activation(out=osb_q[:, o, :], in_=ps[:, :P], func=Ident,
                                 bias=bout[:, o:o + 1], scale=1.0)
            nc.sync.dma_start_transpose(otok_q[:, o * P:(o + 1) * P],
                                        osb_q[:, o, :])
            osl = slice(o * P, (o + 1) * P)
            nc.vector.tensor_tensor(out=x2[:, qtl, osl],
                                    in0=xt[:, qtl + 1, osl],
                                    in1=otok_q[:, osl],
                                    op=mybir.AluOpType.add)
        _layernorm_tile(nc, small, x2[:, qtl, :], z16[:, qtl, :], eps_t[:])
        nc.sync.dma_start_transpose(zt[:, :, qtl * P:(qtl + 1) * P],
                                    z16[:, qtl, :])

    # ---- FFN ----
    ht = act.tile([P, 32, OWN], BF16, tag="big_xh")     # reuses x slot
    for o in range(32):
        w = wst.tile([P, 8, P], BF16, tag="w_small")
        nc.scalar.dma_start(out=w[:], in_=wff1_t[o])
        ps = psum.tile([P, OWN], F32, tag="ps_big")
        for q in range(NQT):
            for k in range(8):
                nc.tensor.matmul(ps[:, q * P:(q + 1) * P], w[:, k, :],
                                 zt[:, k, q * P:(q + 1) * P],
                                 start=(k == 0), stop=(k == 7))
        nc.scalar.activation(out=ht[:, o, :], in_=ps[:], func=Gelu,
                             bias=bff1[:, o:o + 1], scale=1.0)
    fsb = act.tile([P, 8, OWN], BF16, tag="fsb")        # 8KB/p
    ftok = act.tile([P, NQT, D], BF16, tag="big_zf")    # reuses z16 slot
    yout = act.tile([P, NQT, D], F32, tag="big_qoy")    # reuses qt slot
    for o in range(8):
        w = wst.tile([P, 32, P], BF16, tag="w_ff2")
        nc.sync.dma_start(out=w[:], in_=wff2_t[o])
        ps = psum.tile([P, OWN], F32, tag="ps_big")
        for k in range(32):
            nc.tensor.matmul(ps[:], w[:, k, :], ht[:, k, :],
                             start=(k == 0), stop=(k == 31))
        nc.scalar.activation(out=fsb[:, o, :], in_=ps[:], func=Ident,
                             bias=bff2[:, o:o + 1], scale=1.0)
        nc.sync.dma_start_transpose(ftok[:, :, o * P:(o + 1) * P], fsb[:, o, :])
        osl = slice(o * P, (o + 1) * P)
        for q in range(NQT):
            nc.vector.tensor_tensor(out=yout[:, q, osl], in0=x2[:, q, osl],
                                    in1=ftok[:, q, osl],
                                    op=mybir.AluOpType.add)
    for q in range(NQT):
        nc.sync.dma_start(out=y[q * P:(q + 1) * P, :], in_=yout[:, q, :])
    ctx.close()


def _host_prep(x, ln1_g, ln1_b, w_qkv, w_out, b_out, ln2_g, ln2_b,
               w_ff1, b_ff1, w_ff2, b_ff2):
    """Fold LN affine params into weights, pre-transpose/tile, build per-core
    input maps."""
    f8 = np.float64
    wqkv_eff = (w_qkv.astype(f8) * ln1_g.astype(f8)[None, :])
    bqkv_eff = w_qkv.astype(f8) @ ln1_b.astype(f8)
    wq, wk, wv = wqkv_eff[:INNER], wqkv_eff[INNER:2 * INNER], wqkv_eff[2 * INNER:]
    bq_v, bk_v, bv_v = bqkv_eff[:INNER], bqkv_eff[INNER:2 * INNER], bqkv_eff[2 * INNER:]
    bout_eff = b_out.astype(f8) + w_out.astype(f8) @ bv_v
    wff1_eff = w_ff1.astype(f8) * ln2_g.astype(f8)[None, :]
    bff1_eff = b_ff1.astype(f8) + w_ff1.astype(f8) @ ln2_b.astype(f8)

    def lhst(w, ko, no):  # w [K, N] -> [no, 128, ko, 128] bf16 (p = K within tile)
        a = np.ascontiguousarray(
            w.reshape(ko, P, no, P).transpose(2, 1, 0, 3)).astype(BF)
        return a

    wq_t = lhst(wq.T, 8, 8)
    wk_t = lhst(wk.T, 8, 8)
    wv_t = np.ascontiguousarray(wv.T.reshape(8, P, INNER).transpose(1, 0, 2)).astype(BF)
    wout_t = lhst(w_out.astype(f8).T, 8, 8)
    wff1_t = lhst(wff1_eff.T, 8, 32)
    wff2_t = lhst(w_ff2.astype(f8).T, 32, 8)

    def colmaj(b, n):  # [n*128] -> [128, n] fp32
        return np.ascontiguousarray(b.reshape(n, P).T).astype(np.float32)

    bq_t = colmaj(bq_v, 8)
    bk_t = colmaj(bk_v, 8)
    bout_t = colmaj(bout_eff, 8)
    bff1_t = colmaj(bff1_eff, 32)
    bff2_t = colmaj(b_ff2.astype(f8), 8)

    xf = x.reshape(B * S, D).astype(np.float32)
    in_maps = []
    for c in range(NCORES):
        b = c // (NCORES // B)
        s0 = (c % (NCORES // B)) * OWN          # within-batch start of own rows
        lo, hi = s0 - HALO, s0 + OWN + HALO
        xs = np.zeros((TLOC, D), np.float32)
        clo, chi = max(lo, 0), min(hi, S)
        xs[clo - lo:chi - lo] = xf[b * S + clo:b * S + chi]
        # additive mask [NQT, 128, 384]: query r in tile qtl, key col ccol
        q_idx = s0 + np.arange(OWN)             # within-batch query positions
        mask = np.zeros((NQT, P, KW), np.float32)
        for qtl in range(NQT):
            qq = q_idx[qtl * P:(qtl + 1) * P][:, None]      # [128,1]
            kk = (s0 + qtl * P - HALO) + np.arange(KW)[None, :]
            bad = (np.abs(kk - qq) > WINDOW) | (kk < 0) | (kk >= S)
            mask[qtl][bad] = MASK_NEG
        in_maps.append(dict(
            x_s=xs, maskin=mask, wq_t=wq_t, wk_t=wk_t, wv_t=wv_t,
            wout_t=wout_t, wff1_t=wff1_t, wff2_t=wff2_t,
            bq_t=bq_t, bk_t=bk_t, bout_t=bout_t, bff1_t=bff1_t, bff2_t=bff2_t))
    return in_maps


_NC_CACHE = {}


def kernel(x, ln1_g, ln1_b, w_qkv, w_out, b_out, ln2_g, ln2_b,
           w_ff1, b_ff1, w_ff2, b_ff2, _trace=False):
    in_maps = _host_prep(x, ln1_g, ln1_b, w_qkv, w_out, b_out,
                         ln2_g, ln2_b, w_ff1, b_ff1, w_ff2, b_ff2)
    if "nc" not in _NC_CACHE:
        _NC_CACHE["nc"] = _build_nc()
    nc = _NC_CACHE["nc"]
    res = run_bass_kernel_spmd(nc, in_maps, core_ids=list(range(NCORES)),
                               trace=_trace)
    if _trace:
        _NC_CACHE["last"] = res
    out = np.concatenate([res.results[c]["y"] for c in range(NCORES)], axis=0)
    return out.reshape(B, S, D).astype(np.float32)


# revision 28
# speedup vs baseline: 1.0338x; 1.0103x over previous
"""Trainium2 Bass kernel for nn_ASAMLayer (local-window sparse attention layer).

Sharding: token-parallel across 8 cores. 4096 tokens total -> 512 own tokens
per core, plus a 128-token halo on each side (within-batch, zero-padded at
batch edges) so the WINDOW=128 local attention needs no collectives.

On-chip layout: feature-major ("transposed") activations for all GEMMs.
LayerNorm gains/biases are folded into the following weight matrices on the
host (exact), softmax runs without max-subtraction (scores are bounded),
row-sums come free from the activation engine's accum_out, and the band mask
is a precomputed additive -1e6 tensor applied to scores in PSUM.
"""

import sys

import numpy as np

sys.path.insert(0, "/opt/trn_rl_repo")

import ml_dtypes  # noqa: E402

import concourse.bass as bass  # noqa: E402
from concourse import bacc  # noqa: E402
import concourse.mybir as mybir  # noqa: E402
import concourse.tile as tile  # noqa: E402
from concourse.bass_utils import run_bass_kernel_spmd  # noqa: E402

# Pin Exp and Ln to the joint set and Gelu to its anchor set so the act-table
# load pass emits few loads instead of thrashing per qtile. Set indices are
# preserved (unwanted sets are emptied, not removed).
import concourse.bacc as _bacc_mod  # noqa: E402
import concourse.hw_specs as _hw_specs  # noqa: E402

_orig_get_tables = _hw_specs.get_activation_tables


def _pinned_tables(module_arch):
    t = dict(_orig_get_tables(module_arch))
    keep = {"natural_log_exp_and_others", "gelu_and_others"}
    drop = {mybir.ActivationFunctionType.Exp,
            mybir.ActivationFunctionType.Ln,
            mybir.ActivationFunctionType.Gelu}
    return {name: (fns if name in keep else {f for f in fns if f not in drop})
            for name, fns in t.items()}


_bacc_mod.get_activation_tables = _pinned_tables


B, S, D = 2, 2048, 1024
H, DH = 16, 64
INNER = H * DH          # 1024
FF = 4 * D              # 4096
WINDOW = 128
EPS = 1e-5
SCALE = DH ** -0.5

NCORES = 8
OWN = (B * S) // NCORES          # 512 own tokens per core
HALO = WINDOW                    # 128
TLOC = OWN + 2 * HALO            # 768 local rows (halo'd)
P = 128
NQT = OWN // P                   # 4 query tiles
NKT = TLOC // P                  # 6 key tiles
KW = 3 * P                       # 384-wide key window per query tile

F32 = mybir.dt.float32
BF16 = mybir.dt.bfloat16
BF = ml_dtypes.bfloat16

MASK_NEG = -1.0e6


def _build_nc():
    nc = bacc.Bacc()

    x_s = nc.declare_dram_parameter("x_s", [TLOC, D], F32, isOutput=False)
    maskin = nc.declare_dram_parameter("maskin", [NQT, P, KW], F32, isOutput=False)
    wq_t = nc.declare_dram_parameter("wq_t", [8, P, 8, P], BF16, isOutput=False)
    wk_t = nc.declare_dram_parameter("wk_t", [8, P, 8, P], BF16, isOutput=False)
    wv_t = nc.declare_dram_parameter("wv_t", [P, 8, INNER], BF16, isOutput=False)
    wout_t = nc.declare_dram_parameter("wout_t", [8, P, 8, P], BF16, isOutput=False)
    wff1_t = nc.declare_dram_parameter("wff1_t", [32, P, 8, P], BF16, isOutput=False)
    wff2_t = nc.declare_dram_parameter("wff2_t", [8, P, 32, P], BF16, isOutput=False)
    bq_t = nc.declare_dram_parameter("bq_t", [P, 8], F32, isOutput=False)
    bk_t = nc.declare_dram_parameter("bk_t", [P, 8], F32, isOutput=False)
    bout_t = nc.declare_dram_parameter("bout_t", [P, 8], F32, isOutput=False)
    bff1_t = nc.declare_dram_parameter("bff1_t", [P, 32], F32, isOutput=False)
    bff2_t = nc.declare_dram_parameter("bff2_t", [P, 8], F32, isOutput=False)
    y = nc.declare_dram_parameter("y", [OWN, D], F32, isOutput=True)

    with tile.TileContext(nc) as tc:
        _emit(tc, nc, x_s, maskin, wq_t, wk_t, wv_t, wout_t, wff1_t, wff2_t,
              bq_t, bk_t, bout_t, bff1_t, bff2_t, y)
    nc.finalize()
    return nc


def _layernorm_tile(nc, pool, x_ap, out_bf16, eps_ap):
    """out_bf16 = (x - mean(x)) / sqrt(var(x) + EPS), bf16. x_ap [P, D] fp32."""
    xg = x_ap.rearrange("p (s f) -> p s f", f=512)
    stats = pool.tile([P, 2, 6], F32, tag="ln_stats")
    for s in range(2):
        nc.vector.bn_stats(out=stats[:, s, :], in_=xg[:, s, :])
    mv = pool.tile([P, 2], F32, tag="ln_mv")
    nc.vector.bn_aggr(out=mv[:], in_=stats[:])
    rstd = pool.tile([P, 1], F32, tag="ln_rstd")
    nc.scalar.activation(out=rstd[:], in_=mv[:, 1:2],
                         func=mybir.ActivationFunctionType.Ln,
                         bias=eps_ap, scale=1.0)
    nc.scalar.activation(out=rstd[:], in_=rstd[:],
                         func=mybir.ActivationFunctionType.Exp,
                         scale=-0.5)
    nc.vector.tensor_scalar(
        out=out_bf16[:], in0=x_ap,
        scalar1=mv[:, 0:1], scalar2=rstd[:],
        op0=mybir.AluOpType.subtract, op1=mybir.AluOpType.mult)


def _emit(tc, nc, x_s, maskin, wq_t, wk_t, wv_t, wout_t, wff1_t, wff2_t,
          bq_t, bk_t, bout_t, bff1_t, bff2_t, y):
    from contextlib import ExitStack
    ctx = ExitStack()
    Gelu = mybir.ActivationFunctionType.Gelu
    Exp = mybir.ActivationFunctionType.Exp
    Ident = mybir.ActivationFunctionType.Identity

    const = ctx.enter_context(tc.tile_pool(name="const", bufs=1))
    small = ctx.enter_context(tc.tile_pool(name="small", bufs=4))
    wst = ctx.enter_context(tc.tile_pool(name="wst", bufs=3))
    act = ctx.enter_context(tc.tile_pool(name="act", bufs=1))
    trans = ctx.enter_context(tc.tile_pool(name="trans", bufs=3))
    psum = ctx.enter_context(tc.tile_pool(name="psum", bufs=3, space="PSUM"))
    psum_kw = ctx.enter_context(tc.tile_pool(name="pskw", bufs=3, space="PSUM"))
    psum_small = ctx.enter_context(tc.tile_pool(name="psA", bufs=2, space="PSUM"))

    # ---- load wv (needed first), x, biases, masks ----
    wv = act.tile([P, 8, INNER], BF16, tag="big_wx2")   # 16KB/p (dies after V)
    nc.gpsimd.dma_start(out=wv[:], in_=wv_t[:])
    xt = act.tile([P, NKT, D], F32, tag="big_xh")       # 24KB/p
    for t in range(NKT):
        qeng = nc.sync if t < 3 else nc.scalar
        qeng.dma_start(out=xt[:, t, :], in_=x_s[t * P:(t + 1) * P, :])
    bq = const.tile([P, 8], F32, tag="bq")
    nc.gpsimd.dma_start(out=bq[:], in_=bq_t[:])
    bk = const.tile([P, 8], F32, tag="bk")
    nc.gpsimd.dma_start(out=bk[:], in_=bk_t[:])
    bout = const.tile([P, 8], F32, tag="bout")
    nc.gpsimd.dma_start(out=bout[:], in_=bout_t[:])
    bff1 = const.tile([P, 32], F32, tag="bff1")
    nc.gpsimd.dma_start(out=bff1[:], in_=bff1_t[:])
    bff2 = const.tile([P, 8], F32, tag="bff2")
    nc.gpsimd.dma_start(out=bff2[:], in_=bff2_t[:])
    eps_t = const.tile([P, 1], F32, tag="eps")
    nc.vector.memset(eps_t[:], EPS)
    maskt = act.tile([P, NQT, KW], F32, tag="mask")     # 6KB/p
    for q in range(NQT):
        nc.gpsimd.dma_start(out=maskt[:, q, :], in_=maskin[q])

    # ---- per token-tile: LN1 -> transpose -> V matmuls (PE starts early) ----
    yt = act.tile([P, 8, TLOC], BF16, tag="big_ya")     # 12KB/p
    vtok = act.tile([P, NKT, INNER], BF16, tag="vtok")  # 12KB/p
    for t in range(NKT):
        y16 = trans.tile([P, D], BF16, tag="y16t")
        _layernorm_tile(nc, small, xt[:, t, :], y16[:], eps_t[:])
        nc.sync.dma_start_transpose(yt[:, :, t * P:(t + 1) * P], y16[:])
        for half in range(2):
            ps = psum.tile([P, OWN], F32, tag="ps_big")
            sl = slice(half * 512, (half + 1) * 512)
            for k in range(8):
                nc.tensor.matmul(ps[:], yt[:, k, t * P:(t + 1) * P], wv[:, k, sl],
                                 start=(k == 0), stop=(k == 7))
            nc.vector.tensor_copy(out=vtok[:, t, sl], in_=ps[:])

    # ---- Q (own queries) and K (all local keys), feature-major ----
    qt_sb = act.tile([P, 8, OWN], BF16, tag="big_qoy")  # 8KB/p
    kt_sb = act.tile([P, 8, TLOC], BF16, tag="kt")      # 12KB/p
    for o in range(8):
        w = wst.tile([P, 8, P], BF16, tag="w_small")
        nc.sync.dma_start(out=w[:], in_=wq_t[o])
        ps = psum.tile([P, OWN], F32, tag="ps_big")
        for k in range(8):
            nc.tensor.matmul(ps[:], w[:, k, :], yt[:, k, HALO:HALO + OWN],
                             start=(k == 0), stop=(k == 7))
        nc.scalar.activation(out=qt_sb[:, o, :], in_=ps[:], func=Ident,
                             bias=bq[:, o:o + 1], scale=1.0)
    for o in range(8):
        w = wst.tile([P, 8, P], BF16, tag="w_small")
        nc.sync.dma_start(out=w[:], in_=wk_t[o])
        for half in range(2):
            ps = psum_kw.tile([P, KW], F32, tag="ps_kw")
            sl = slice(half * KW, (half + 1) * KW)
            for k in range(8):
                nc.tensor.matmul(ps[:], w[:, k, :], yt[:, k, sl],
                                 start=(k == 0), stop=(k == 7))
            nc.scalar.activation(out=kt_sb[:, o, sl], in_=ps[:], func=Ident,
                                 bias=bk[:, o:o + 1], scale=1.0)

    wout_sb = act.tile([P, 8, 8, P], BF16, tag="woutr")   # 16KB/p resident
    for o in range(8):
        nc.sync.dma_start(out=wout_sb[:, o], in_=wout_t[o])

    # ---- attention + per-qtile epilogue (out-proj, residual, LN2, Zt) ----
    avt = act.tile([P, 8, OWN], BF16, tag="big_ya")     # reuses yt slot
    x2 = act.tile([P, NQT, D], F32, tag="big_wx2")      # reuses wv slot
    z16 = act.tile([P, NQT, D], BF16, tag="big_zf")     # 8KB/p
    zt = act.tile([P, 8, OWN], BF16, tag="zt")          # 8KB/p
    for qtl in range(NQT):
        qsl = slice(qtl * P, (qtl + 1) * P)
        for i in range(8):
            for hh in range(2):
                h = 2 * i + hh
                hs = slice(hh * DH, (hh + 1) * DH)
                ksl = slice(qtl * P, qtl * P + KW)
                sc = psum_kw.tile([P, KW], F32, tag="ps_kw")
                nc.tensor.matmul(sc[:], qt_sb[hs, i, qsl], kt_sb[hs, i, ksl],
                                 start=True, stop=True)
                scm = trans.tile([P, KW], F32, tag="scm")
                nc.vector.tensor_tensor(out=scm[:], in0=sc[:],
                                        in1=maskt[:, qtl, :],
                                        op=mybir.AluOpType.add)
                ptm = trans.tile([P, KW], BF16, tag="ptm")
                ssum = small.tile([P, 1], F32, tag="ssum")
                nc.scalar.activation(out=ptm[:], in_=scm[:], func=Exp,
                                     scale=SCALE, accum_out=ssum[:])
                nc.vector.reciprocal(out=ssum[:], in_=ssum[:])
                nc.vector.tensor_scalar_mul(ptm[:], ptm[:], ssum[:])
                ptt = trans.tile([P, 3, P], BF16, tag="ptt")
                for e in range(3):
                    nc.sync.dma_start_transpose(ptt[:, e, :],
                                                ptm[:, e * P:(e + 1) * P])
                av = psum_small.tile([DH, P], F32, tag="ps_av")
                for e in range(3):
                    nc.tensor.matmul(av[:], vtok[:, qtl + e, h * DH:(h + 1) * DH],
                                     ptt[:, e, :], start=(e == 0), stop=(e == 2))
                nc.any.tensor_copy(out=avt[hs, i, qsl], in_=av[:])
        # out-projection for this qtile, then residual + LN2 + Zt
        osb_q = trans.tile([P, 8, P], BF16, tag="osbq")
        otok_q = trans.tile([P, D], BF16, tag="otokq")
        for o in range(8):
            ps = psum_kw.tile([P, KW], F32, tag="ps_kw")
            for k in range(8):
                nc.tensor.matmul(ps[:, :P], wout_sb[:, o, k, :], avt[:, k, qsl],
                                 start=(k == 0), stop=(k == 7))
            nc.scalar.activation(out=osb_q[:, o, :], in_=ps[:, :P], func=Ident,
                                 bias=bout[:, o:o + 1], scale=1.0)
            nc.sync.dma_start_transpose(otok_q[:, o * P:(o + 1) * P],
                                        osb_q[:, o, :])
            osl = slice(o * P, (o + 1) * P)
            nc.vector.tensor_tensor(out=x2[:, qtl, osl],
                                    in0=xt[:, qtl + 1, osl],
                                    in1=otok_q[:, osl],
                                    op=mybir.AluOpType.add)
        _layernorm_tile(nc, small, x2[:, qtl, :], z16[:, qtl, :], eps_t[:])
        nc.sync.dma_start_transpose(zt[:, :, qtl * P:(qtl + 1) * P],
                                    z16[:, qtl, :])

    # ---- FFN ----
    ht = act.tile([P, 32, OWN], BF16, tag="big_xh")     # reuses x slot
    for o in range(32):
        w = wst.tile([P, 8, P], BF16, tag="w_small")
        nc.scalar.dma_start(out=w[:], in_=wff1_t[o])
        ps = psum.tile([P, OWN], F32, tag="ps_big")
        for q in range(NQT):
            for k in range(8):
                nc.tensor.matmul(ps[:, q * P:(q + 1) * P], w[:, k, :],
                                 zt[:, k, q * P:(q + 1) * P],
                                 start=(k == 0), stop=(k == 7))
        nc.scalar.activation(out=ht[:, o, :], in_=ps[:], func=Gelu,
                             bias=bff1[:, o:o + 1], scale=1.0)
    fsb = act.tile([P, 8, OWN], BF16, tag="fsb")        # 8KB/p
    ftok = act.tile([P, NQT, D], BF16, tag="big_zf")    # reuses z16 slot
    yout = act.tile([P, NQT, D], F32, tag="big_qoy")    # reuses qt slot
    for o in range(8):
        w = wst.tile([P, 32, P], BF16, tag="w_ff2")
        nc.sync.dma_start(out=w[:], in_=wff2_t[o])
        ps = psum.tile([P, OWN], F32, tag="ps_big")
        for k in range(32):
            nc.tensor.matmul(ps[:], w[:, k, :], ht[:, k, :],
                             start=(k == 0), stop=(k == 31))
        nc.scalar.activation(out=fsb[:, o, :], in_=ps[:], func=Ident,
                             bias=bff2[:, o:o + 1], scale=1.0)
        nc.sync.dma_start_transpose(ftok[:, :, o * P:(o + 1) * P], fsb[:, o, :])
        osl = slice(o * P, (o + 1) * P)
        for q in range(NQT):
            nc.vector.tensor_tensor(out=yout[:, q, osl], in0=x2[:, q, osl],
                                    in1=ftok[:, q, osl],
                                    op=mybir.AluOpType.add)
    for q in range(NQT):
        nc.sync.dma_start(out=y[q * P:(q + 1) * P, :], in_=yout[:, q, :])
    ctx.close()


def _host_prep(x, ln1_g, ln1_b, w_qkv, w_out, b_out, ln2_g, ln2_b,
               w_ff1, b_ff1, w_ff2, b_ff2):
    """Fold LN affine params into weights, pre-transpose/tile, build per-core
    input maps."""
    f8 = np.float64
    wqkv_eff = (w_qkv.astype(f8) * ln1_g.astype(f8)[None, :])
    bqkv_eff = w_qkv.astype(f8) @ ln1_b.astype(f8)
    wq, wk, wv = wqkv_eff[:INNER], wqkv_eff[INNER:2 * INNER], wqkv_eff[2 * INNER:]
    bq_v, bk_v, bv_v = bqkv_eff[:INNER], bqkv_eff[INNER:2 * INNER], bqkv_eff[2 * INNER:]
    bout_eff = b_out.astype(f8) + w_out.astype(f8) @ bv_v
    wff1_eff = w_ff1.astype(f8) * ln2_g.astype(f8)[None, :]
    bff1_eff = b_ff1.astype(f8) + w_ff1.astype(f8) @ ln2_b.astype(f8)

    def lhst(w, ko, no):  # w [K, N] -> [no, 128, ko, 128] bf16 (p = K within tile)
        a = np.ascontiguousarray(
            w.reshape(ko, P, no, P).transpose(2, 1, 0, 3)).astype(BF)
        return a

    wq_t = lhst(wq.T, 8, 8)
    wk_t = lhst(wk.T, 8, 8)
    wv_t = np.ascontiguousarray(wv.T.reshape(8, P, INNER).transpose(1, 0, 2)).astype(BF)
    wout_t = lhst(w_out.astype(f8).T, 8, 8)
    wff1_t = lhst(wff1_eff.T, 8, 32)
    wff2_t = lhst(w_ff2.astype(f8).T, 32, 8)

    def colmaj(b, n):  # [n*128] -> [128, n] fp32
        return np.ascontiguousarray(b.reshape(n, P).T).astype(np.float32)

    bq_t = colmaj(bq_v, 8)
    bk_t = colmaj(bk_v, 8)
    bout_t = colmaj(bout_eff, 8)
    bff1_t = colmaj(bff1_eff, 32)
    bff2_t = colmaj(b_ff2.astype(f8), 8)

    xf = x.reshape(B * S, D).astype(np.float32)
    in_maps = []
    for c in range(NCORES):
        b = c // (NCORES // B)
        s0 = (c % (NCORES // B)) * OWN          # within-batch start of own rows
        lo, hi = s0 - HALO, s0 + OWN + HALO
        xs = np.zeros((TLOC, D), np.float32)
        clo, chi = max(lo, 0), min(hi, S)
        xs[clo - lo:chi - lo] = xf[b * S + clo:b * S + chi]
        # additive mask [NQT, 128, 384]: query r in tile qtl, key col ccol
        q_idx = s0 + np.arange(OWN)             # within-batch query positions
        mask = np.zeros((NQT, P, KW), np.float32)
        for qtl in range(NQT):
            qq = q_idx[qtl * P:(qtl + 1) * P][:, None]      # [128,1]
            kk = (s0 + qtl * P - HALO) + np.arange(KW)[None, :]
            bad = (np.abs(kk - qq) > WINDOW) | (kk < 0) | (kk >= S)
            mask[qtl][bad] = MASK_NEG
        in_maps.append(dict(
            x_s=xs, maskin=mask, wq_t=wq_t, wk_t=wk_t, wv_t=wv_t,
            wout_t=wout_t, wff1_t=wff1_t, wff2_t=wff2_t,
            bq_t=bq_t, bk_t=bk_t, bout_t=bout_t, bff1_t=bff1_t, bff2_t=bff2_t))
    return in_maps


_NC_CACHE = {}


def kernel(x, ln1_g, ln1_b, w_qkv, w_out, b_out, ln2_g, ln2_b,
           w_ff1, b_ff1, w_ff2, b_ff2, _trace=False):
    in_maps = _host_prep(x, ln1_g, ln1_b, w_qkv, w_out, b_out,
                         ln2_g, ln2_b, w_ff1, b_ff1, w_ff2, b_ff2)
    if "nc" not in _NC_CACHE:
        _NC_CACHE["nc"] = _build_nc()
    nc = _NC_CACHE["nc"]
    res = run_bass_kernel_spmd(nc, in_maps, core_ids=list(range(NCORES)),
                               trace=_trace)
    if _trace:
        _NC_CACHE["last"] = res
    out = np.concatenate([res.results[c]["y"] for c in range(NCORES)], axis=0)
    return out.reshape(B, S, D).astype(np.float32)
